# revision 1
# baseline (speedup 1.0000x reference)
"""Cross-attention kernel for TRN2, data-parallel over batch (B=8) on 8 cores.

Reference computation per batch element:
    xt  = proj_in(x)              # [L=4096, E=512], 1x1 conv == matmul
    Q   = xt @ W_q.T + b_q
    K   = ctx @ W_k.T + b_k       # ctx: [S=1024, E]
    V   = ctx @ W_v.T + b_v
    att = softmax(Q @ K.T * scale)
    out = proj_out((att @ V).T)   # [C=512, 64, 64]

Host-side algebraic folds (exact up to fp rounding):
  * scale, W_pi, W_q, W_k fold into a single matrix on the Q/K path:
      G = (scale * W_q @ W_pi).T @ W_k ;  logits.T = (G @ ctx).T-contract X
    (the Q'.b_k rank-1 term is constant across keys -> softmax-invariant,
    dropped; a nonzero bias path reappears as per-partition q0 on GC^T X)
  * W_v and W_po fold:  WV = (W_po @ W_v).T ; b_o = b_po + W_po @ b_v
  * softmax normalization is applied at the very end (divide by Z), so the
    attention core is exp -> matmul -> scale-by-1/Z.

On-device, the data-dependent weight products are built ONCE per core
(GC = G.T-contract ctx, VW = ctx.T-contract WV), then every query chunk
needs only the two unavoidable attention GEMMs plus the Z column-sum:
  ST[j,i] = GC.T-contract X ; PT = exp(ST)
  Z[i]    = ones.T @ PT (column sums via PE)
  U[o,i]  = VW.T-contract PT
  y[o,i]  = U * (1/Z broadcast via GpSimd) + b_o

All matmuls run in fp32r (TF32-like: 11-bit mantissa, low 12 bits zero).
DRAM-sourced matmul operands are pre-rounded on the host; device-produced
operands (GC, VW, PT) are rounded by the producing engine via an fp32r
output dtype. PSUM accumulation stays full fp32.
"""

import numpy as np

import concourse.bass as bass
import concourse.mybir as mybir
import concourse.tile as tile
from concourse import bacc
from concourse.bass_utils import run_bass_kernel_spmd

F32 = mybir.dt.float32
F32R = mybir.dt.float32r
EXP = mybir.ActivationFunctionType.Exp

C = 512       # in channels
E = 512       # emb dim
L = 4096      # query length (64*64)
S = 1024      # key length (32*32)
LI = 512      # i-chunk (query) tile size
NCHUNK = L // LI
NCORES = 8

TRACE = False           # test harness can flip this before calling kernel()
LAST_RESULTS = None     # stashed BassKernelResults for the test harness

_PROGRAM_CACHE = {}


def _round_tf32(a: np.ndarray) -> np.ndarray:
    """Round fp32 mantissa to 11 explicit bits (round-to-nearest-even),
    zeroing the low 12 bits — the fp32r operand format."""
    a = np.ascontiguousarray(a, dtype=np.float32)
    b = a.view(np.uint32)
    r = (b + np.uint32(0x7FF) + ((b >> np.uint32(12)) & np.uint32(1))) & np.uint32(
        0xFFFFF000
    )
    return r.view(np.float32)


def _build_program(has_q0: bool, has_bo: bool):
    nc = bacc.Bacc(
        "TRN2",
        target_bir_lowering=False,
        debug=False,
        enable_asserts=False,
        num_devices=NCORES,
    )
    x_d = nc.dram_tensor("x", [C, L], F32R, kind="ExternalInput").ap()
    ctx_d = nc.dram_tensor("ctx", [E, S], F32R, kind="ExternalInput").ap()
    # gt arrives host-permuted into ct-major blocks: gt_d[p, ct*512+et*128+c']
    # = G.T[et*128+p, ct*128+c'], so the first GC group (ct=0) only needs the
    # first 256KB block and DMA runs stay 2KB-contiguous.
    gt_d = nc.dram_tensor("gt", [128, 4 * C], F32R, kind="ExternalInput").ap()
    wv_d = nc.dram_tensor("wv", [E, E], F32R, kind="ExternalInput").ap()
    onec_d = nc.dram_tensor("onec", [128, 1], F32R, kind="ExternalInput").ap()
    q0_d = bo_d = None
    if has_q0:
        q0_d = nc.dram_tensor("q0", [128, 8], F32, kind="ExternalInput").ap()
    if has_bo:
        bo_d = nc.dram_tensor("bo", [128, 4], F32, kind="ExternalInput").ap()
    y_d = nc.dram_tensor("y", [C, L], F32, kind="ExternalOutput").ap()

    def load_4stack(pool, dram_ap, width, name):
        """[4*128, width] DRAM -> [128, 4*width] SBUF tile (partition-chunk t
        lands at free offset t*width). One DMA per chunk so consumers of an
        individual chunk can start as soon as that chunk lands (subtile
        deps), and the four transfers spread across DMA queues."""
        t = pool.tile([128, 4 * width], F32R, name=name, tag=name)
        for tt in range(4):
            nc.sync.dma_start(
                t[:, tt * width:(tt + 1) * width],
                dram_ap[tt * 128:(tt + 1) * 128, :],
            )
        return t

    with tile.TileContext(nc) as tc:
        from contextlib import ExitStack

        with ExitStack() as ctx:
            cpool = ctx.enter_context(tc.tile_pool(name="consts", bufs=1))
            ps_s = ctx.enter_context(tc.tile_pool(name="ps_s", bufs=3, space="PSUM"))
            ps_z = ctx.enter_context(tc.tile_pool(name="ps_z", bufs=1, space="PSUM"))
            ps_u = ctx.enter_context(tc.tile_pool(name="ps_u", bufs=4, space="PSUM"))
            xpool = ctx.enter_context(tc.tile_pool(name="xp", bufs=2))
            ppool = ctx.enter_context(tc.tile_pool(name="pp", bufs=2))
            opool = ctx.enter_context(tc.tile_pool(name="op", bufs=2))
            zpool = ctx.enter_context(tc.tile_pool(name="zp", bufs=2))

            # ---- loads in latency-priority order --------------------------
            ones_col = cpool.tile([128, 1], F32R, name="ones_col")
            nc.sync.dma_start(ones_col[:], onec_d[:, :])
            # interleave gt chunks with ctx first-halves so the jh=0 GC
            # groups (which need gt[*] + ctx[*, :LI]) complete after ~2MB of
            # DMA instead of the full 3MB; ctx second-halves follow X0.
            GTS = cpool.tile([128, 4 * C], F32R, name="gstk", tag="gstk")
            CTXT = cpool.tile([128, 4 * S], F32R, name="cstk", tag="cstk")
            nc.sync.dma_start(GTS[:, 0:512], gt_d[:, 0:512])
            for tt in range(4):
                nc.sync.dma_start(
                    CTXT[:, tt * S:tt * S + LI],
                    ctx_d[tt * 128:(tt + 1) * 128, 0:LI],
                )
            for ctb in range(1, 4):
                nc.sync.dma_start(
                    GTS[:, ctb * 512:(ctb + 1) * 512], gt_d[:, ctb * 512:(ctb + 1) * 512]
                )

            def load_x(ic):
                xt = xpool.tile([128, 4 * LI], F32R, name="xc", tag="x")
                nc.sync.dma_start(
                    xt[:].rearrange("p (t c) -> p t c", c=LI),
                    x_d[:, bass.ts(ic, LI)].rearrange("(t p) c -> p t c", p=128),
                )
                return xt

            X0 = load_x(0)                                        # prefetch chunk 0
            for tt in range(4):
                nc.sync.dma_start(
                    CTXT[:, tt * S + LI:(tt + 1) * S],
                    ctx_d[tt * 128:(tt + 1) * 128, LI:S],
                )
            WVT = load_4stack(cpool, wv_d[:, :], E, "wstk")       # [128, 4*E]
            q0_s = bo_s = None
            if has_q0:
                q0_s = cpool.tile([128, 8], F32, name="q0s")
                nc.sync.dma_start(q0_s[:], q0_d[:, :])
            if has_bo:
                bo_s = cpool.tile([128, 4], F32, name="bos")
                nc.sync.dma_start(bo_s[:], bo_d[:, :])

            def ctx_blk(et, jt):            # CTX [e-chunk et, j-tile jt]
                return CTXT[:, et * S + jt * 128: et * S + (jt + 1) * 128]

            # ---- GC[c, j] = sum_e G[c, e] ctx[e, j]  (Q/K path, once) ----
            # jh-outer: the four jh=0 groups need only the ctx first-halves,
            # which are the first DMAs to land.
            GC = [
                cpool.tile([128, S], F32R, name=f"gc{ct}", tag=f"gc{ct}")
                for ct in range(4)
            ]
            for jh in range(2):
                for ct in range(4):
                    gps = ps_s.tile([128, LI], F32, name="gps", tag="s")
                    for et in range(4):
                        nc.tensor.matmul(
                            gps[:],
                            GTS[:, ct * 512 + et * 128: ct * 512 + (et + 1) * 128],
                            CTXT[:, et * S + jh * LI: et * S + (jh + 1) * LI],
                            start=(et == 0),
                            stop=(et == 3),
                        )
                    nc.vector.tensor_copy(GC[ct][:, jh * LI:(jh + 1) * LI], gps[:])

            X = X0
            for ic in range(NCHUNK):
                isl = bass.ts(ic, LI)
                Xc = X
                if ic + 1 < NCHUNK:
                    X = load_x(ic + 1)      # prefetch next chunk
                # ST[j, i] = GC.T-contract X (+ q0[j]) ; PT = exp(ST).
                # The Z partial-sum tree (pairwise adds on the DVE) is
                # interleaved into the S-loop so each add issues as soon as
                # its exp operands exist; the final 128-partition fold is a
                # single ones-matmul placed after the first U-group so the
                # in-order PE stream never waits on the DVE.
                PT = []
                tpart = {}
                for jt in range(8):
                    sps = ps_s.tile([128, LI], F32, name="sps", tag="s")
                    for ct in range(4):
                        nc.tensor.matmul(
                            sps[:],
                            GC[ct][:, jt * 128:(jt + 1) * 128],
                            Xc[:, bass.ts(ct, LI)],
                            start=(ct == 0),
                            stop=(ct == 3),
                        )
                    p = ppool.tile([128, LI], F32R, name="pt", tag=f"p{jt}")
                    if has_q0:
                        nc.scalar.activation(
                            p[:], sps[:], EXP, bias=q0_s[:, jt:jt + 1]
                        )
                    else:
                        nc.scalar.activation(p[:], sps[:], EXP)
                    PT.append(p)
                    if jt in (1, 3, 5, 7):
                        t = zpool.tile([128, LI], F32, name="tp", tag=f"t{jt // 2}")
                        nc.vector.tensor_add(
                            t[:],
                            PT[jt - 1][:].bitcast(F32),
                            PT[jt][:].bitcast(F32),
                        )
                        tpart[jt // 2] = t
                    if jt == 3:
                        ta = zpool.tile([128, LI], F32, name="ta", tag="ta")
                        nc.vector.tensor_add(ta[:], tpart[0][:], tpart[1][:])
                    if jt == 7:
                        tb = zpool.tile([128, LI], F32, name="tb", tag="tb")
                        nc.vector.tensor_add(tb[:], tpart[2][:], tpart[3][:])
                        zt = zpool.tile([128, LI], F32R, name="zt", tag="zt")
                        nc.vector.tensor_add(zt[:], ta[:], tb[:])
                if ic == 0:
                    # VW[j, o] = sum_e ctx[e, j] WV[e, o] (V/out path, once).
                    # Emitted between chunk 0's S and U stages so it hides in
                    # the exp/Z latency instead of delaying the first chunk.
                    VW = []
                    for jt in range(8):
                        vps = ps_s.tile([128, E], F32, name="vps", tag="s")
                        for et in range(4):
                            nc.tensor.matmul(
                                vps[:],
                                ctx_blk(et, jt),
                                WVT[:, bass.ts(et, E)],
                                start=(et == 0),
                                stop=(et == 3),
                            )
                        vw = cpool.tile(
                            [128, E], F32R, name=f"vwt{jt}", tag=f"vwt{jt}"
                        )
                        nc.vector.tensor_copy(vw[:], vps[:])
                        VW.append(vw)
                # U[o, i] = VW.T-contract PT ; y = U * invZ (+ b_o)
                for ot in range(4):
                    ups = ps_u.tile([128, LI], F32, name="ups", tag="u")
                    for jt in range(8):
                        nc.tensor.matmul(
                            ups[:],
                            VW[jt][:, ot * 128:(ot + 1) * 128],
                            PT[jt][:],
                            start=(jt == 0),
                            stop=(jt == 7),
                        )
                    if ot == 0:
                        zps = ps_z.tile([1, LI], F32, name="zps", tag="z")
                        nc.tensor.matmul(
                            zps[:], ones_col[:], zt[:], start=True, stop=True
                        )
                        invz = zpool.tile([1, LI], F32, name="invz", tag="invz")
                        # full-precision reciprocal costs 3.3us on the DVE and
                        # gates the output chain; the fast approx (~18 correct
                        # bits, well beyond the fp32r noise floor) is ~5x
                        # faster. Z is strictly positive so the undefined
                        # edge cases (0/denorm/inf) cannot occur.
                        nc.vector.reciprocal_approx_fast(out=invz[:], in_=zps[:])
                        invz_rep = zpool.tile(
                            [128, LI], F32, name="invz_rep", tag="invzrep"
                        )
                        nc.gpsimd.partition_broadcast(invz_rep[:], invz[:])
                    o = opool.tile([128, LI], F32, name="ot", tag=f"o{ot}")
                    nc.vector.tensor_mul(o[:], ups[:], invz_rep[:])
                    if has_bo:
                        nc.vector.tensor_scalar_add(o[:], o[:], bo_s[:, ot:ot + 1])
                    nc.sync.dma_start(y_d[ot * 128:(ot + 1) * 128, isl], o[:])

    nc.compile()
    return nc


def kernel(**inputs) -> np.ndarray:
    global LAST_RESULTS
    x = np.asarray(inputs["x"], dtype=np.float32)
    context = np.asarray(inputs["context"], dtype=np.float32)
    W_pi = np.asarray(inputs["W_pi"], dtype=np.float64)
    b_pi = np.asarray(inputs["b_pi"], dtype=np.float64)
    W_q = np.asarray(inputs["W_q"], dtype=np.float64)
    b_q = np.asarray(inputs["b_q"], dtype=np.float64)
    W_k = np.asarray(inputs["W_k"], dtype=np.float64)
    W_v = np.asarray(inputs["W_v"], dtype=np.float64)
    b_v = np.asarray(inputs["b_v"], dtype=np.float64)
    W_po = np.asarray(inputs["W_po"], dtype=np.float64)
    b_po = np.asarray(inputs["b_po"], dtype=np.float64)

    scale = float(E) ** -0.5
    Wqpi = scale * (W_q @ W_pi)                            # [dq, c]
    G = (Wqpi.T @ W_k)                                     # [c, e]
    GT = _round_tf32(np.ascontiguousarray(G.T).astype(np.float32))  # [e, c]
    # ct-major block permutation: A[p, ct*512+et*128+c'] = GT[et*128+p, ct*128+c']
    GT = np.ascontiguousarray(
        GT.reshape(4, 128, 4, 128).transpose(1, 2, 0, 3).reshape(128, 4 * C)
    )
    b_row = scale * (W_q @ b_pi + b_q)
    # per-KEY bias on the logits: q0[j] = (W_k.T b_row) . ctx[:, j] is handled
    # as an activation bias per j-partition, computed from ctx on the host
    # would be data-work; instead fold the e-space bias through the device GC
    # path is impossible (it multiplies ctx), so compute the per-j bias here
    # only when biases are actually nonzero (they are all zero in this
    # problem's inputs).
    q0_e = (W_k.T @ b_row).astype(np.float64)              # [e]
    WV = _round_tf32((W_po @ W_v).T.astype(np.float32))    # [e, o]
    b_o = (b_po + W_po @ b_v).astype(np.float32)           # [o]

    has_q0 = bool(np.any(q0_e))
    has_bo = bool(np.any(b_o))
    key = (has_q0, has_bo)
    if key not in _PROGRAM_CACHE:
        _PROGRAM_CACHE[key] = _build_program(has_q0, has_bo)
    nc = _PROGRAM_CACHE[key]

    ones_c = np.ones((128, 1), dtype=np.float32)
    in_maps = []
    for c in range(NCORES):
        ctx_mat = context[c].reshape(E, S)
        m = {
            "x": _round_tf32(x[c].reshape(C, L)),
            "ctx": _round_tf32(ctx_mat),
            "gt": GT,
            "wv": WV,
            "onec": ones_c,
        }
        if has_q0:
            # logits bias per key j: q0_e . ctx[:, j]  -> [S] -> [128, 8]
            q0j = (q0_e @ ctx_mat.astype(np.float64)).astype(np.float32)
            m["q0"] = np.ascontiguousarray(q0j.reshape(8, 128).T)
        if has_bo:
            m["bo"] = np.ascontiguousarray(b_o.reshape(4, 128).T)
        in_maps.append(m)

    res = run_bass_kernel_spmd(nc, in_maps, core_ids=list(range(NCORES)), trace=TRACE)
    LAST_RESULTS = res
    y = np.stack([res.results[c]["y"] for c in range(NCORES)], axis=0)
    return np.ascontiguousarray(y.reshape(NCORES, C, 64, 64).astype(np.float32))



# revision 6
# speedup vs baseline: 1.0622x; 1.0622x over previous
"""Cross-attention kernel for TRN2, data-parallel over batch (B=8) on 8 cores.

Reference computation per batch element:
    xt  = proj_in(x)              # [L=4096, E=512], 1x1 conv == matmul
    Q   = xt @ W_q.T + b_q
    K   = ctx @ W_k.T + b_k       # ctx: [S=1024, E]
    V   = ctx @ W_v.T + b_v
    att = softmax(Q @ K.T * scale)
    out = proj_out((att @ V).T)   # [C=512, 64, 64]

Host-side algebraic folds (weights only, exact up to fp rounding):
  * scale, W_pi, W_q, W_k fold into G = (scale * W_q @ W_pi).T @ W_k, so
    logits.T = (G.T-contract ctx).T-contract X.  G is pre-scaled by 2^kg
    (weights-only bound) so its fp8 products stay in e4m3 normal range;
    the inverse scale rides the exp() activation's scale operand.
  * W_v and W_po fold:  WV = (W_po @ W_v).T ; b_o = b_po + W_po @ b_v

The two big attention GEMMs (logits ST = GC.T-c X and output U = VW.T-c PT)
run as fp8e4m3 DoubleRow matmuls: 256 contraction rows per instruction at
0.5 cycles/row -- 2x the bf16/fp32r MAC rate.  fp8's 3-bit mantissa cannot
represent softmax weights P ~= 1 +- 0.1 (quantization step 0.125 at 1.0),
so the softmax is mean-split:  P = 1 + p,  U = Vbar + sum_j p_j VW_j with
p = exp(s) - 1 cast to fp8 (full relative resolution on the deviation) and
Vbar = sum_j VW_j computed via a bf16 path (cbar = rowsum(ctx);
Vbar = cbar @ WV) so the dominant mean term carries no fp8 noise.
Z = 1024 + sum_j p_j from a DoubleRow ones-matmul over the same p8 tiles
(consistent with the numerator).

Per-core precomputes GC = G.T-c ctx and VW = ctx.T-c WV stay bf16 (fp8
operands there cost ~0.5e-2 extra max-norm error for only ~7us).

Measured end-to-end max-norm rel err of this quantization pipeline vs the
fp64 reference: ~1.1e-2 (budget 2e-2).
"""

import numpy as np
import ml_dtypes

import concourse.bass as bass
import concourse.mybir as mybir
import concourse.tile as tile
from concourse import bacc
from concourse.bass_utils import run_bass_kernel_spmd

F32 = mybir.dt.float32
BF16 = mybir.dt.bfloat16
F8 = mybir.dt.float8e4
EXP = mybir.ActivationFunctionType.Exp
DR = mybir.MatmulPerfMode.DoubleRow
AXX = mybir.AxisListType.X
ADD = mybir.AluOpType.add

NP_F8 = ml_dtypes.float8_e4m3
NP_BF = ml_dtypes.bfloat16

C = 512       # in channels
E = 512       # emb dim
L = 4096      # query length (64*64)
S = 1024      # key length (32*32)
LI = 512      # i-chunk (query) tile size
NCHUNK = L // LI
NCORES = 8

TRACE = False           # test harness can flip this before calling kernel()
LAST_RESULTS = None     # stashed BassKernelResults for the test harness

_PROGRAM_CACHE = {}


def _two(ap, inner):
    """[128, 2*inner] AP -> [128, 2, inner] for DoubleRow operands."""
    return ap.rearrange("p (two n) -> p two n", two=2, n=inner)


def _build_program(has_q0: bool, has_bo: bool):
    nc = bacc.Bacc(
        "TRN2",
        target_bir_lowering=False,
        debug=False,
        enable_asserts=False,
        num_devices=NCORES,
    )
    x_d = nc.dram_tensor("x", [C, L], F8, kind="ExternalInput").ap()
    ctx_d = nc.dram_tensor("ctx", [E, S], BF16, kind="ExternalInput").ap()
    # gt arrives host-permuted into ct-major blocks: gt_d[p, ct*512+et*128+c']
    # = (2^kg * G.T)[et*128+p, ct*128+c'].
    gt_d = nc.dram_tensor("gt", [128, 4 * C], BF16, kind="ExternalInput").ap()
    wv_d = nc.dram_tensor("wv", [E, E], BF16, kind="ExternalInput").ap()
    # [128, 32] of ones; the Sigma-p DoubleRow lhsT reads cols {0, 16} so the
    # pair stride is 16B (dual-fp8 ldweights requires 16B-aligned even steps).
    one8_d = nc.dram_tensor("one8", [128, 32], F8, kind="ExternalInput").ap()
    one16_d = nc.dram_tensor("one16", [1, LI], BF16, kind="ExternalInput").ap()
    sc_d = nc.dram_tensor("sc", [128, 1], F32, kind="ExternalInput").ap()
    q0_d = bo_d = None
    if has_q0:
        q0_d = nc.dram_tensor("q0", [128, 8], F32, kind="ExternalInput").ap()
    if has_bo:
        bo_d = nc.dram_tensor("bo", [128, 4], F32, kind="ExternalInput").ap()
    y_d = nc.dram_tensor("y", [C, L], BF16, kind="ExternalOutput").ap()

    with tile.TileContext(nc) as tc:
        from contextlib import ExitStack

        with ExitStack() as ctx:
            cpool = ctx.enter_context(tc.tile_pool(name="consts", bufs=1))
            ps_s = ctx.enter_context(tc.tile_pool(name="ps_s", bufs=3, space="PSUM"))
            ps_z = ctx.enter_context(tc.tile_pool(name="ps_z", bufs=1, space="PSUM"))
            ps_u = ctx.enter_context(tc.tile_pool(name="ps_u", bufs=4, space="PSUM"))
            xpool = ctx.enter_context(tc.tile_pool(name="xp", bufs=2))
            ppool = ctx.enter_context(tc.tile_pool(name="pp", bufs=2))
            p8pool = ctx.enter_context(tc.tile_pool(name="p8p", bufs=2))
            opool = ctx.enter_context(tc.tile_pool(name="op", bufs=2))
            zpool = ctx.enter_context(tc.tile_pool(name="zp", bufs=2))

            # ---- loads in latency-priority order --------------------------
            one8_s = cpool.tile([128, 32], F8, name="one8s")
            nc.sync.dma_start(one8_s[:], one8_d[:, :])
            one16_s = cpool.tile([1, LI], BF16, name="one16s")
            nc.sync.dma_start(one16_s[:], one16_d[:, :])
            sc_s = cpool.tile([128, 1], F32, name="scs")
            nc.sync.dma_start(sc_s[:], sc_d[:, :])
            # gt ct=0 + ctx first halves unblock the jh=0 GC groups early.
            GTS = cpool.tile([128, 4 * C], BF16, name="gstk", tag="gstk")
            CTXT = cpool.tile([128, 4 * S], BF16, name="cstk", tag="cstk")
            nc.sync.dma_start(GTS[:, 0:512], gt_d[:, 0:512])
            for tt in range(4):
                nc.sync.dma_start(
                    CTXT[:, tt * S:tt * S + LI],
                    ctx_d[tt * 128:(tt + 1) * 128, 0:LI],
                )
            for ctb in range(1, 4):
                nc.sync.dma_start(
                    GTS[:, ctb * 512:(ctb + 1) * 512],
                    gt_d[:, ctb * 512:(ctb + 1) * 512],
                )

            def load_x(ic):
                xt = xpool.tile([128, 4 * LI], F8, name="xc", tag="x")
                nc.sync.dma_start(
                    xt[:].rearrange("p (t c) -> p t c", c=LI),
                    x_d[:, bass.ts(ic, LI)].rearrange("(t p) c -> p t c", p=128),
                )
                return xt

            X0 = load_x(0)                                        # prefetch chunk 0
            for tt in range(4):
                nc.sync.dma_start(
                    CTXT[:, tt * S + LI:(tt + 1) * S],
                    ctx_d[tt * 128:(tt + 1) * 128, LI:S],
                )
            WVT = cpool.tile([128, 4 * E], BF16, name="wstk", tag="wstk")
            for tt in range(4):
                nc.sync.dma_start(
                    WVT[:, tt * E:(tt + 1) * E],
                    wv_d[tt * 128:(tt + 1) * 128, :],
                )
            q0_s = bo_s = None
            if has_q0:
                q0_s = cpool.tile([128, 8], F32, name="q0s")
                nc.sync.dma_start(q0_s[:], q0_d[:, :])
            if has_bo:
                bo_s = cpool.tile([128, 4], F32, name="bos")
                nc.sync.dma_start(bo_s[:], bo_d[:, :])

            # ---- GC[c, j] = sum_e G[c, e] ctx[e, j]  (bf16, once) ---------
            # Output goes straight to the DoubleRow-interleaved fp8 layout:
            # GCD[cp][p, jt*256 + t*128 + m] = GC[(2cp+t)*128+p, jt*128+m].
            GCD = [
                cpool.tile([128, 2048], F8, name=f"gcd{cp}", tag=f"gcd{cp}")
                for cp in range(2)
            ]
            for jh in range(2):
                for ct in range(4):
                    gps = ps_s.tile([128, LI], F32, name="gps", tag="s")
                    for et in range(4):
                        nc.tensor.matmul(
                            gps[:],
                            GTS[:, ct * 512 + et * 128: ct * 512 + (et + 1) * 128],
                            CTXT[:, et * S + jh * LI: et * S + (jh + 1) * LI],
                            start=(et == 0),
                            stop=(et == 3),
                        )
                    dst = GCD[ct // 2][:, jh * 1024:(jh + 1) * 1024].rearrange(
                        "p (j two m) -> p j two m", two=2, m=128
                    )[:, :, ct % 2, :]
                    nc.vector.tensor_copy(
                        dst, gps[:].rearrange("p (j m) -> p j m", m=128)
                    )

            # ---- cbar[e] = sum_j ctx[e, j] ; Vbar = cbar @ WV (bf16) ------
            cbar = cpool.tile([128, 4], F32, name="cbar")
            nc.vector.tensor_reduce(
                cbar[:], CTXT[:].rearrange("p (t j) -> p t j", t=4), AXX, ADD
            )
            cbar16 = cpool.tile([128, 4], BF16, name="cbar16")
            nc.vector.tensor_copy(cbar16[:], cbar[:])

            def st_group(ic, jt, X, p8cur):
                """ST[j,i] for one j-tile: 2 DoubleRow fp8 matmuls, exp on
                scalar (with the 2^-kg descale), p8 = P - 1 cast on DVE."""
                sps = ps_s.tile([128, LI], F32, name="sps", tag="s")
                nc.tensor.matmul(
                    sps[:],
                    _two(GCD[0][:, jt * 256:(jt + 1) * 256], 128),
                    _two(X[:, 0:2 * LI], LI),
                    start=True,
                    stop=False,
                    perf_mode=DR,
                )
                nc.tensor.matmul(
                    sps[:],
                    _two(GCD[1][:, jt * 256:(jt + 1) * 256], 128),
                    _two(X[:, 2 * LI:4 * LI], LI),
                    start=False,
                    stop=True,
                    perf_mode=DR,
                )
                p = ppool.tile([128, LI], F32, name="pt", tag=f"p{jt}")
                if has_q0:
                    nc.scalar.activation(
                        p[:], sps[:], EXP,
                        bias=q0_s[:, jt:jt + 1], scale=sc_s[:, 0:1],
                    )
                else:
                    nc.scalar.activation(p[:], sps[:], EXP, scale=sc_s[:, 0:1])
                jp, t = jt // 2, jt % 2
                if t == 0:
                    p8cur[jp] = p8pool.tile(
                        [128, 2 * LI], F8, name=f"p8_{jp}", tag=f"p8_{jp}"
                    )
                nc.vector.tensor_scalar_add(
                    p8cur[jp][:, t * LI:(t + 1) * LI], p[:], -1.0
                )

            def zsum_emit(p8prev):
                """Z - 1024 = sum_j p_j via DoubleRow ones-matmul, then the
                invz chain (DVE add/recip, gpsimd partition broadcast)."""
                zps = ps_z.tile([1, LI], F32, name="zps", tag="z")
                for jp in range(4):
                    nc.tensor.matmul(
                        zps[:],
                        _two(one8_s[:], 16)[:, :, 0:1],
                        _two(p8prev[jp][:], LI),
                        start=(jp == 0),
                        stop=(jp == 3),
                        perf_mode=DR,
                    )
                zr = zpool.tile([1, LI], F32, name="zr", tag="zr")
                nc.vector.tensor_scalar_add(zr[:], zps[:], 1024.0)
                invz = zpool.tile([1, LI], F32, name="invz", tag="invz")
                nc.vector.reciprocal_approx_fast(out=invz[:], in_=zr[:])
                invz_rep = zpool.tile([128, LI], F32, name="invzr", tag="invzr")
                nc.gpsimd.partition_broadcast(invz_rep[:], invz[:])
                return invz_rep

            def u_group(ic, ot, p8prev, VW8D, v16, invz_rep):
                """U[o,i] = Vbar[o] + sum_j p_j VW[j,o] (psum), then
                y = U * invz (DVE, bf16 out) and DMA out."""
                ups = ps_u.tile([128, LI], F32, name="ups", tag="u")
                nc.tensor.matmul(
                    ups[:],
                    v16[:, ot * 128:(ot + 1) * 128],
                    one16_s[:],
                    start=True,
                    stop=False,
                )
                for jp in range(4):
                    nc.tensor.matmul(
                        ups[:],
                        _two(VW8D[jp][:], E)[:, :, ot * 128:(ot + 1) * 128],
                        _two(p8prev[jp][:], LI),
                        start=False,
                        stop=(jp == 3),
                        perf_mode=DR,
                    )
                o = opool.tile([128, LI], BF16, name="ot", tag=f"o{ot}")
                nc.vector.tensor_mul(o[:], ups[:], invz_rep[:])
                if has_bo:
                    nc.vector.tensor_scalar_add(o[:], o[:], bo_s[:, ot:ot + 1])
                nc.sync.dma_start(y_d[ot * 128:(ot + 1) * 128, bass.ts(ic, LI)], o[:])

            # ---- window 0: ST(0), then VW + Vbar precompute ---------------
            X = X0
            Xnext = load_x(1)
            p8cur = {}
            st_group(0, 0, X, p8cur)
            st_group(0, 1, X, p8cur)
            # VW[j, o] = sum_e ctx[e, j] WV[e, o] (bf16, once), emitted after
            # the first ST groups so chunk 0's exp pipeline starts early; the
            # remaining ST groups interleave so sps psum slots recycle.
            VW8D = [None] * 4
            for jt in range(8):
                vps = ps_s.tile([128, E], F32, name="vps", tag="s")
                for et in range(4):
                    nc.tensor.matmul(
                        vps[:],
                        CTXT[:, et * S + jt * 128: et * S + (jt + 1) * 128],
                        WVT[:, et * E:(et + 1) * E],
                        start=(et == 0),
                        stop=(et == 3),
                    )
                jp, t = jt // 2, jt % 2
                if t == 0:
                    VW8D[jp] = cpool.tile(
                        [128, 2 * E], F8, name=f"vw8_{jp}", tag=f"vw8_{jp}"
                    )
                nc.vector.tensor_copy(VW8D[jp][:, t * E:(t + 1) * E], vps[:])
                if jt < 6:
                    st_group(0, jt + 2, X, p8cur)
            vb = ps_s.tile([1, E], F32, name="vb", tag="s")
            for et in range(4):
                nc.tensor.matmul(
                    vb[:],
                    cbar16[:, et:et + 1],
                    WVT[:, et * E:(et + 1) * E],
                    start=(et == 0),
                    stop=(et == 3),
                )
            v16 = cpool.tile([1, E], BF16, name="v16")
            nc.vector.tensor_copy(v16[:], vb[:])

            # ---- windows 1..8: ST(w) interleaved with U(w-1) --------------
            for w in range(1, NCHUNK + 1):
                p8prev, p8cur = p8cur, {}
                X, Xnext = Xnext, (load_x(w + 1) if w + 1 < NCHUNK else None)
                invz_rep = zsum_emit(p8prev)
                for k in range(4):
                    if w < NCHUNK:
                        st_group(w, 2 * k, X, p8cur)
                        st_group(w, 2 * k + 1, X, p8cur)
                    u_group(w - 1, k, p8prev, VW8D, v16, invz_rep)

    nc.compile()
    return nc


def kernel(**inputs) -> np.ndarray:
    global LAST_RESULTS
    x = np.asarray(inputs["x"], dtype=np.float32)
    context = np.asarray(inputs["context"], dtype=np.float32)
    W_pi = np.asarray(inputs["W_pi"], dtype=np.float64)
    b_pi = np.asarray(inputs["b_pi"], dtype=np.float64)
    W_q = np.asarray(inputs["W_q"], dtype=np.float64)
    b_q = np.asarray(inputs["b_q"], dtype=np.float64)
    W_k = np.asarray(inputs["W_k"], dtype=np.float64)
    W_v = np.asarray(inputs["W_v"], dtype=np.float64)
    b_v = np.asarray(inputs["b_v"], dtype=np.float64)
    W_po = np.asarray(inputs["W_po"], dtype=np.float64)
    b_po = np.asarray(inputs["b_po"], dtype=np.float64)

    scale = float(E) ** -0.5
    Wqpi = scale * (W_q @ W_pi)                            # [dq, c]
    G = (Wqpi.T @ W_k)                                     # [c, e]
    # fp8 pre-scale: |GC[c,j]| <= ||G[c,:]|| * ||ctx[:,j]|| and gaussian ctx
    # columns concentrate at sqrt(512)~22.6; 32 is a ~1.4x-margin bound.
    # Target max ~200 (e4m3 max normal is 240).
    rowg = float(np.linalg.norm(G, axis=1).max())
    kg = int(np.floor(np.log2(200.0 / (rowg * 32.0))))
    GT = np.ascontiguousarray(G.T * (2.0 ** kg)).astype(np.float32)   # [e, c]
    # ct-major block permutation: A[p, ct*512+et*128+c'] = GT[et*128+p, ct*128+c']
    GT = np.ascontiguousarray(
        GT.reshape(4, 128, 4, 128).transpose(1, 2, 0, 3).reshape(128, 4 * C)
    ).astype(NP_BF)
    b_row = scale * (W_q @ b_pi + b_q)
    q0_e = (W_k.T @ b_row).astype(np.float64)              # [e]
    WV = ((W_po @ W_v).T).astype(np.float32).astype(NP_BF)  # [e, o]
    b_o = (b_po + W_po @ b_v).astype(np.float32)           # [o]

    has_q0 = bool(np.any(q0_e))
    has_bo = bool(np.any(b_o))
    key = (has_q0, has_bo)
    if key not in _PROGRAM_CACHE:
        _PROGRAM_CACHE[key] = _build_program(has_q0, has_bo)
    nc = _PROGRAM_CACHE[key]

    one8 = np.ones((128, 32), dtype=NP_F8)
    one16 = np.ones((1, LI), dtype=NP_BF)
    sc = np.full((128, 1), 2.0 ** -kg, dtype=np.float32)
    in_maps = []
    for c in range(NCORES):
        ctx_mat = context[c].reshape(E, S)
        m = {
            "x": x[c].reshape(C, L).astype(NP_F8),
            "ctx": ctx_mat.astype(NP_BF),
            "gt": GT,
            "wv": WV,
            "one8": one8,
            "one16": one16,
            "sc": sc,
        }
        if has_q0:
            # logits bias per key j: q0_e . ctx[:, j]  -> [S] -> [128, 8]
            q0j = (q0_e @ ctx_mat.astype(np.float64)).astype(np.float32)
            m["q0"] = np.ascontiguousarray(q0j.reshape(8, 128).T)
        if has_bo:
            m["bo"] = np.ascontiguousarray(b_o.reshape(4, 128).T)
        in_maps.append(m)

    res = run_bass_kernel_spmd(nc, in_maps, core_ids=list(range(NCORES)), trace=TRACE)
    LAST_RESULTS = res
    y = np.stack(
        [np.asarray(res.results[c]["y"]).astype(np.float32) for c in range(NCORES)],
        axis=0,
    )
    return np.ascontiguousarray(y.reshape(NCORES, C, 64, 64))


# revision 12
# speedup vs baseline: 1.2473x; 1.1743x over previous
"""Cross-attention kernel for TRN2, data-parallel over batch (B=8) on 8 cores.

Reference computation per batch element:
    xt  = proj_in(x)              # [L=4096, E=512], 1x1 conv == matmul
    Q   = xt @ W_q.T + b_q
    K   = ctx @ W_k.T + b_k       # ctx: [S=1024, E]
    V   = ctx @ W_v.T + b_v
    att = softmax(Q @ K.T * scale)
    out = proj_out((att @ V).T)   # [C=512, 64, 64]

Host-side algebraic folds (weights only, exact up to fp rounding):
  * scale, W_pi, W_q, W_k fold into G = (scale * W_q @ W_pi).T @ W_k, so
    logits.T = (G.T-contract ctx).T-contract X.  G is pre-scaled by 2^kg
    (weights-only bound) so its fp8 products stay in e4m3 normal range;
    the inverse scale rides the exp() activation's scale operand.
  * W_v and W_po fold:  WV = (W_po @ W_v).T ; b_o = b_po + W_po @ b_v

The two big attention GEMMs (logits ST = GC.T-c X and output U = VW.T-c PT)
run as fp8e4m3 DoubleRow matmuls: 256 contraction rows per instruction at
0.5 cycles/row -- 2x the bf16/fp32r MAC rate.  fp8's 3-bit mantissa cannot
represent softmax weights P ~= 1 +- 0.1 (quantization step 0.125 at 1.0),
so the softmax is mean-split:  P = 1 + p,  U = Vbar + sum_j p_j VW_j with
p = exp(s) - 1 cast to fp8 (full relative resolution on the deviation) and
Vbar = sum_j VW_j computed via a bf16 path (cbar = rowsum(ctx);
Vbar = cbar @ WV) so the dominant mean term carries no fp8 noise.
Z = 1024 + sum_j p_j from a DoubleRow ones-matmul over the same p8 tiles
(consistent with the numerator).

Per-core precomputes GC = G.T-c ctx and VW = ctx.T-c WV stay bf16 (fp8
operands there cost ~0.5e-2 extra max-norm error for only ~7us).

Measured end-to-end max-norm rel err of this quantization pipeline vs the
fp64 reference: ~1.1e-2 (budget 2e-2).
"""

import numpy as np
import ml_dtypes

import concourse.bass as bass
import concourse.mybir as mybir
import concourse.tile as tile
from concourse import bacc
from concourse.bass_utils import run_bass_kernel_spmd

F32 = mybir.dt.float32
BF16 = mybir.dt.bfloat16
F8 = mybir.dt.float8e4
EXP = mybir.ActivationFunctionType.Exp
DR = mybir.MatmulPerfMode.DoubleRow
AXX = mybir.AxisListType.X
ADD = mybir.AluOpType.add

NP_F8 = ml_dtypes.float8_e4m3
NP_BF = ml_dtypes.bfloat16

C = 512       # in channels
E = 512       # emb dim
L = 4096      # query length (64*64)
S = 1024      # key length (32*32)
LI = 512      # i-chunk (query) tile size
NCHUNK = L // LI
NCORES = 8

TRACE = False           # test harness can flip this before calling kernel()
LAST_RESULTS = None     # stashed BassKernelResults for the test harness

_PROGRAM_CACHE = {}


def _two(ap, inner):
    """[128, 2*inner] AP -> [128, 2, inner] for DoubleRow operands."""
    return ap.rearrange("p (two n) -> p two n", two=2, n=inner)


def _build_program(has_q0: bool, has_bo: bool, kg: int):
    nc = bacc.Bacc(
        "TRN2",
        target_bir_lowering=False,
        debug=False,
        enable_asserts=False,
        num_devices=NCORES,
    )
    x_d = nc.dram_tensor("x", [C, L], F8, kind="ExternalInput").ap()
    ctx_d = nc.dram_tensor("ctx", [E, S], BF16, kind="ExternalInput").ap()
    # gt arrives host-permuted into ct-major blocks: gt_d[p, ct*512+et*128+c']
    # = (2^kg * G.T)[et*128+p, ct*128+c'].
    gt_d = nc.dram_tensor("gt", [128, 4 * C], BF16, kind="ExternalInput").ap()
    wv_d = nc.dram_tensor("wv", [E, E], BF16, kind="ExternalInput").ap()
    # [128, 32] of ones; the Sigma-p DoubleRow lhsT reads cols {0, 16} so the
    # pair stride is 16B (dual-fp8 ldweights requires 16B-aligned even steps).
    one8_d = nc.dram_tensor("one8", [128, 32], F8, kind="ExternalInput").ap()
    one16_d = nc.dram_tensor("one16", [1, LI], BF16, kind="ExternalInput").ap()
    q0_d = bo_d = None
    if has_q0:
        q0_d = nc.dram_tensor("q0", [128, 8], F32, kind="ExternalInput").ap()
    if has_bo:
        bo_d = nc.dram_tensor("bo", [128, 4], F32, kind="ExternalInput").ap()
    y_d = nc.dram_tensor("y", [C, L], BF16, kind="ExternalOutput").ap()

    with tile.TileContext(nc) as tc:
        from contextlib import ExitStack

        with ExitStack() as ctx:
            cpool = ctx.enter_context(tc.tile_pool(name="consts", bufs=1))
            ps_s = ctx.enter_context(tc.tile_pool(name="ps_s", bufs=3, space="PSUM"))
            ps_z = ctx.enter_context(tc.tile_pool(name="ps_z", bufs=1, space="PSUM"))
            ps_u = ctx.enter_context(tc.tile_pool(name="ps_u", bufs=4, space="PSUM"))
            xpool = ctx.enter_context(tc.tile_pool(name="xp", bufs=2))
            ppool = ctx.enter_context(tc.tile_pool(name="pp", bufs=2))
            p8pool = ctx.enter_context(tc.tile_pool(name="p8p", bufs=2))
            opool = ctx.enter_context(tc.tile_pool(name="op", bufs=2))
            zpool = ctx.enter_context(tc.tile_pool(name="zp", bufs=2))

            # ---- loads in latency-priority order --------------------------
            one8_s = cpool.tile([128, 32], F8, name="one8s")
            nc.sync.dma_start(one8_s[:], one8_d[:, :])
            one16_s = cpool.tile([1, LI], BF16, name="one16s")
            nc.sync.dma_start(one16_s[:], one16_d[:, :])
            scale_imm = float(2.0 ** -kg)
            # gt ct=0 + ctx first halves unblock the jh=0 GC groups early.
            GTS = cpool.tile([128, 4 * C], BF16, name="gstk", tag="gstk")
            CTXT = cpool.tile([128, 4 * S], BF16, name="cstk", tag="cstk")
            nc.sync.dma_start(GTS[:, 0:512], gt_d[:, 0:512])
            for tt in range(4):
                nc.sync.dma_start(
                    CTXT[:, tt * S:tt * S + LI],
                    ctx_d[tt * 128:(tt + 1) * 128, 0:LI],
                )
            for ctb in range(1, 4):
                nc.sync.dma_start(
                    GTS[:, ctb * 512:(ctb + 1) * 512],
                    gt_d[:, ctb * 512:(ctb + 1) * 512],
                )

            def load_x(ic):
                xt = xpool.tile([128, 4 * LI], F8, name="xc", tag="x")
                nc.sync.dma_start(
                    xt[:].rearrange("p (t c) -> p t c", c=LI),
                    x_d[:, bass.ts(ic, LI)].rearrange("(t p) c -> p t c", p=128),
                )
                return xt

            X0 = load_x(0)                                        # prefetch chunk 0
            for tt in range(4):
                nc.sync.dma_start(
                    CTXT[:, tt * S + LI:(tt + 1) * S],
                    ctx_d[tt * 128:(tt + 1) * 128, LI:S],
                )
            WVT = cpool.tile([128, 4 * E], BF16, name="wstk", tag="wstk")
            for tt in range(4):
                nc.sync.dma_start(
                    WVT[:, tt * E:(tt + 1) * E],
                    wv_d[tt * 128:(tt + 1) * 128, :],
                )
            q0_s = bo_s = None
            if has_q0:
                q0_s = cpool.tile([128, 8], F32, name="q0s")
                nc.sync.dma_start(q0_s[:], q0_d[:, :])
            if has_bo:
                bo_s = cpool.tile([128, 4], F32, name="bos")
                nc.sync.dma_start(bo_s[:], bo_d[:, :])

            # ---- GC[c, j] = sum_e G[c, e] ctx[e, j]  (bf16, once) ---------
            # Output goes straight to the DoubleRow-interleaved fp8 layout:
            # GCD[cp][p, jt*256 + t*128 + m] = GC[(2cp+t)*128+p, jt*128+m].
            GCD = [
                cpool.tile([128, 2048], F8, name=f"gcd{cp}", tag=f"gcd{cp}")
                for cp in range(2)
            ]
            for jh in range(2):
                for ct in range(4):
                    gps = ps_s.tile([128, LI], F32, name="gps", tag="s")
                    for et in range(4):
                        nc.tensor.matmul(
                            gps[:],
                            GTS[:, ct * 512 + et * 128: ct * 512 + (et + 1) * 128],
                            CTXT[:, et * S + jh * LI: et * S + (jh + 1) * LI],
                            start=(et == 0),
                            stop=(et == 3),
                        )
                    dst = GCD[ct // 2][:, jh * 1024:(jh + 1) * 1024].rearrange(
                        "p (j two m) -> p j two m", two=2, m=128
                    )[:, :, ct % 2, :]
                    nc.vector.tensor_copy(
                        dst, gps[:].rearrange("p (j m) -> p j m", m=128)
                    )

            # ---- cbar[e] = sum_j ctx[e, j] ; Vbar = cbar @ WV (bf16) ------
            cbar = cpool.tile([128, 4], F32, name="cbar")
            nc.vector.tensor_reduce(
                cbar[:], CTXT[:].rearrange("p (t j) -> p t j", t=4), AXX, ADD
            )
            cbar16 = cpool.tile([128, 4], BF16, name="cbar16")
            nc.vector.tensor_copy(cbar16[:], cbar[:])

            def st_group(ic, jt, X, p8cur):
                """ST[j,i] for one j-tile: 2 DoubleRow fp8 matmuls, exp on
                scalar (with the 2^-kg descale), p8 = P - 1 cast on DVE."""
                sps = ps_s.tile([128, LI], F32, name="sps", tag="s")
                nc.tensor.matmul(
                    sps[:],
                    _two(GCD[0][:, jt * 256:(jt + 1) * 256], 128),
                    _two(X[:, 0:2 * LI], LI),
                    start=True,
                    stop=False,
                    perf_mode=DR,
                )
                nc.tensor.matmul(
                    sps[:],
                    _two(GCD[1][:, jt * 256:(jt + 1) * 256], 128),
                    _two(X[:, 2 * LI:4 * LI], LI),
                    start=False,
                    stop=True,
                    perf_mode=DR,
                )
                p = ppool.tile([128, LI], F32, name="pt", tag=f"p{jt}")
                if has_q0:
                    nc.scalar.activation(
                        p[:], sps[:], EXP,
                        bias=q0_s[:, jt:jt + 1], scale=scale_imm,
                    )
                else:
                    nc.scalar.activation(p[:], sps[:], EXP, scale=scale_imm)
                jp, t = jt // 2, jt % 2
                if t == 0:
                    p8cur[jp] = p8pool.tile(
                        [128, 2 * LI], F8, name=f"p8_{jp}", tag=f"p8_{jp}"
                    )
                nc.vector.tensor_scalar_add(
                    p8cur[jp][:, t * LI:(t + 1) * LI], p[:], -1.0
                )

            def zsum_emit(p8prev):
                """Z - 1024 = sum_j p_j via DoubleRow ones-matmul, then the
                invz chain (DVE add/recip, gpsimd partition broadcast)."""
                zps = ps_z.tile([1, LI], F32, name="zps", tag="z")
                for jp in range(4):
                    nc.tensor.matmul(
                        zps[:],
                        _two(one8_s[:], 16)[:, :, 0:1],
                        _two(p8prev[jp][:], LI),
                        start=(jp == 0),
                        stop=(jp == 3),
                        perf_mode=DR,
                    )
                zr = zpool.tile([1, LI], F32, name="zr", tag="zr")
                nc.vector.tensor_scalar_add(zr[:], zps[:], 1024.0)
                invz = zpool.tile([1, LI], F32, name="invz", tag="invz")
                nc.vector.reciprocal_approx_fast(out=invz[:], in_=zr[:])
                invz_rep = zpool.tile([128, LI], F32, name="invzr", tag="invzr")
                nc.gpsimd.partition_broadcast(invz_rep[:], invz[:])
                return invz_rep

            def u_group(ic, ot, p8prev, VW8D, v16, invz_rep):
                """U[o,i] = Vbar[o] + sum_j p_j VW[j,o] (psum), then
                y = U * invz (DVE, bf16 out) and DMA out."""
                ups = ps_u.tile([128, LI], F32, name="ups", tag="u")
                nc.tensor.matmul(
                    ups[:],
                    v16[:, ot * 128:(ot + 1) * 128],
                    one16_s[:],
                    start=True,
                    stop=False,
                )
                for jp in range(4):
                    nc.tensor.matmul(
                        ups[:],
                        _two(VW8D[jp][:], E)[:, :, ot * 128:(ot + 1) * 128],
                        _two(p8prev[jp][:], LI),
                        start=False,
                        stop=(jp == 3),
                        perf_mode=DR,
                    )
                o = opool.tile([128, LI], BF16, name="ot", tag=f"o{ot}")
                nc.vector.tensor_mul(o[:], ups[:], invz_rep[:])
                if has_bo:
                    nc.vector.tensor_scalar_add(o[:], o[:], bo_s[:, ot:ot + 1])
                nc.sync.dma_start(y_d[ot * 128:(ot + 1) * 128, bass.ts(ic, LI)], o[:])

            # ---- window 0: ST(0), then VW + Vbar precompute ---------------
            X = X0
            Xnext = load_x(1)
            p8cur = {}
            st_group(0, 0, X, p8cur)
            st_group(0, 1, X, p8cur)
            # VW[j, o] = sum_e ctx[e, j] WV[e, o] (bf16, once), emitted after
            # the first ST groups so chunk 0's exp pipeline starts early; the
            # remaining ST groups interleave so sps psum slots recycle.
            VW8D = [None] * 4
            for jt in range(8):
                vps = ps_s.tile([128, E], F32, name="vps", tag="s")
                for et in range(4):
                    nc.tensor.matmul(
                        vps[:],
                        CTXT[:, et * S + jt * 128: et * S + (jt + 1) * 128],
                        WVT[:, et * E:(et + 1) * E],
                        start=(et == 0),
                        stop=(et == 3),
                    )
                jp, t = jt // 2, jt % 2
                if t == 0:
                    VW8D[jp] = cpool.tile(
                        [128, 2 * E], F8, name=f"vw8_{jp}", tag=f"vw8_{jp}"
                    )
                nc.vector.tensor_copy(VW8D[jp][:, t * E:(t + 1) * E], vps[:])
                if jt < 6:
                    st_group(0, jt + 2, X, p8cur)
            vb = ps_s.tile([1, E], F32, name="vb", tag="s")
            for et in range(4):
                nc.tensor.matmul(
                    vb[:],
                    cbar16[:, et:et + 1],
                    WVT[:, et * E:(et + 1) * E],
                    start=(et == 0),
                    stop=(et == 3),
                )
            v16 = cpool.tile([1, E], BF16, name="v16")
            nc.vector.tensor_copy(v16[:], vb[:])

            # ---- windows 1..8: ST(w) interleaved with U(w-1) --------------
            for w in range(1, NCHUNK + 1):
                p8prev, p8cur = p8cur, {}
                X, Xnext = Xnext, (load_x(w + 1) if w + 1 < NCHUNK else None)
                invz_rep = zsum_emit(p8prev)
                for k in range(4):
                    if w < NCHUNK:
                        st_group(w, 2 * k, X, p8cur)
                        st_group(w, 2 * k + 1, X, p8cur)
                    u_group(w - 1, k, p8prev, VW8D, v16, invz_rep)

    nc.compile()
    return nc


def kernel(**inputs) -> np.ndarray:
    global LAST_RESULTS
    x = np.asarray(inputs["x"], dtype=np.float32)
    context = np.asarray(inputs["context"], dtype=np.float32)
    W_pi = np.asarray(inputs["W_pi"], dtype=np.float64)
    b_pi = np.asarray(inputs["b_pi"], dtype=np.float64)
    W_q = np.asarray(inputs["W_q"], dtype=np.float64)
    b_q = np.asarray(inputs["b_q"], dtype=np.float64)
    W_k = np.asarray(inputs["W_k"], dtype=np.float64)
    W_v = np.asarray(inputs["W_v"], dtype=np.float64)
    b_v = np.asarray(inputs["b_v"], dtype=np.float64)
    W_po = np.asarray(inputs["W_po"], dtype=np.float64)
    b_po = np.asarray(inputs["b_po"], dtype=np.float64)

    scale = float(E) ** -0.5
    Wqpi = scale * (W_q @ W_pi)                            # [dq, c]
    G = (Wqpi.T @ W_k)                                     # [c, e]
    # fp8 pre-scale: |GC[c,j]| <= ||G[c,:]|| * ||ctx[:,j]|| and gaussian ctx
    # columns concentrate at sqrt(512)~22.6; 32 is a ~1.4x-margin bound.
    # Target max ~200 (e4m3 max normal is 240).
    rowg = float(np.linalg.norm(G, axis=1).max())
    kg = int(np.floor(np.log2(200.0 / (rowg * 32.0))))
    GT = np.ascontiguousarray(G.T * (2.0 ** kg)).astype(np.float32)   # [e, c]
    # ct-major block permutation: A[p, ct*512+et*128+c'] = GT[et*128+p, ct*128+c']
    GT = np.ascontiguousarray(
        GT.reshape(4, 128, 4, 128).transpose(1, 2, 0, 3).reshape(128, 4 * C)
    ).astype(NP_BF)
    b_row = scale * (W_q @ b_pi + b_q)
    q0_e = (W_k.T @ b_row).astype(np.float64)              # [e]
    WV = ((W_po @ W_v).T).astype(np.float32).astype(NP_BF)  # [e, o]
    b_o = (b_po + W_po @ b_v).astype(np.float32)           # [o]

    has_q0 = bool(np.any(q0_e))
    has_bo = bool(np.any(b_o))
    key = (has_q0, has_bo, kg)
    if key not in _PROGRAM_CACHE:
        _PROGRAM_CACHE[key] = _build_program(has_q0, has_bo, kg)
    nc = _PROGRAM_CACHE[key]

    one8 = np.ones((128, 32), dtype=NP_F8)
    one16 = np.ones((1, LI), dtype=NP_BF)
    in_maps = []
    for c in range(NCORES):
        ctx_mat = context[c].reshape(E, S)
        m = {
            "x": x[c].reshape(C, L).astype(NP_F8),
            "ctx": ctx_mat.astype(NP_BF),
            "gt": GT,
            "wv": WV,
            "one8": one8,
            "one16": one16,
        }
        if has_q0:
            # logits bias per key j: q0_e . ctx[:, j]  -> [S] -> [128, 8]
            q0j = (q0_e @ ctx_mat.astype(np.float64)).astype(np.float32)
            m["q0"] = np.ascontiguousarray(q0j.reshape(8, 128).T)
        if has_bo:
            m["bo"] = np.ascontiguousarray(b_o.reshape(4, 128).T)
        in_maps.append(m)

    res = run_bass_kernel_spmd(nc, in_maps, core_ids=list(range(NCORES)), trace=TRACE)
    LAST_RESULTS = res
    y = np.stack(
        [np.asarray(res.results[c]["y"]).astype(np.float32) for c in range(NCORES)],
        axis=0,
    )
    return np.ascontiguousarray(y.reshape(NCORES, C, 64, 64))


# revision 14
# speedup vs baseline: 1.2496x; 1.0018x over previous
"""Cross-attention kernel for TRN2, data-parallel over batch (B=8) on 8 cores.

Reference computation per batch element:
    xt  = proj_in(x)              # [L=4096, E=512], 1x1 conv == matmul
    Q   = xt @ W_q.T + b_q
    K   = ctx @ W_k.T + b_k       # ctx: [S=1024, E]
    V   = ctx @ W_v.T + b_v
    att = softmax(Q @ K.T * scale)
    out = proj_out((att @ V).T)   # [C=512, 64, 64]

Host-side algebraic folds (weights only, exact up to fp rounding):
  * scale, W_pi, W_q, W_k fold into G = (scale * W_q @ W_pi).T @ W_k, so
    logits.T = (G.T-contract ctx).T-contract X.  G is pre-scaled by 2^kg
    (weights-only bound) so its fp8 products stay in e4m3 normal range;
    the inverse scale rides the exp() activation's scale operand.
  * W_v and W_po fold:  WV = (W_po @ W_v).T ; b_o = b_po + W_po @ b_v

The two big attention GEMMs (logits ST = GC.T-c X and output U = VW.T-c PT)
run as fp8e4m3 DoubleRow matmuls: 256 contraction rows per instruction at
0.5 cycles/row -- 2x the bf16/fp32r MAC rate.  fp8's 3-bit mantissa cannot
represent softmax weights P ~= 1 +- 0.1 (quantization step 0.125 at 1.0),
so the softmax is mean-split:  P = 1 + p,  U = Vbar + sum_j p_j VW_j with
p = exp(s) - 1 cast to fp8 (full relative resolution on the deviation) and
Vbar = sum_j VW_j computed via a bf16 path (cbar = rowsum(ctx);
Vbar = cbar @ WV) so the dominant mean term carries no fp8 noise.
Z = 1024 + sum_j p_j from a DoubleRow ones-matmul over the same p8 tiles
(consistent with the numerator).

Per-core precomputes GC = G.T-c ctx and VW = ctx.T-c WV stay bf16 (fp8
operands there cost ~0.5e-2 extra max-norm error for only ~7us).

Measured end-to-end max-norm rel err of this quantization pipeline vs the
fp64 reference: ~1.1e-2 (budget 2e-2).
"""

import numpy as np
import ml_dtypes

import concourse.bass as bass
import concourse.mybir as mybir
import concourse.tile as tile
from concourse import bacc
from concourse.bass_utils import run_bass_kernel_spmd

F32 = mybir.dt.float32
BF16 = mybir.dt.bfloat16
F8 = mybir.dt.float8e4
EXP = mybir.ActivationFunctionType.Exp
DR = mybir.MatmulPerfMode.DoubleRow
AXX = mybir.AxisListType.X
ADD = mybir.AluOpType.add

NP_F8 = ml_dtypes.float8_e4m3
NP_BF = ml_dtypes.bfloat16

C = 512       # in channels
E = 512       # emb dim
L = 4096      # query length (64*64)
S = 1024      # key length (32*32)
LI = 512      # i-chunk (query) tile size
NCHUNK = L // LI
NCORES = 8

TRACE = False           # test harness can flip this before calling kernel()
LAST_RESULTS = None     # stashed BassKernelResults for the test harness

_PROGRAM_CACHE = {}


def _two(ap, inner):
    """[128, 2*inner] AP -> [128, 2, inner] for DoubleRow operands."""
    return ap.rearrange("p (two n) -> p two n", two=2, n=inner)


def _build_program(has_q0: bool, has_bo: bool, kg: int):
    nc = bacc.Bacc(
        "TRN2",
        target_bir_lowering=False,
        debug=False,
        enable_asserts=False,
        num_devices=NCORES,
    )
    x_d = nc.dram_tensor("x", [C, L], F8, kind="ExternalInput").ap()
    ctx_d = nc.dram_tensor("ctx", [E, S], BF16, kind="ExternalInput").ap()
    # gt arrives host-permuted into ct-major blocks: gt_d[p, ct*512+et*128+c']
    # = (2^kg * G.T)[et*128+p, ct*128+c'].
    gt_d = nc.dram_tensor("gt", [128, 4 * C], BF16, kind="ExternalInput").ap()
    wv_d = nc.dram_tensor("wv", [E, E], BF16, kind="ExternalInput").ap()
    # [128, 32] of ones; the Sigma-p DoubleRow lhsT reads cols {0, 16} so the
    # pair stride is 16B (dual-fp8 ldweights requires 16B-aligned even steps).
    one8_d = nc.dram_tensor("one8", [128, 32], F8, kind="ExternalInput").ap()
    one16_d = nc.dram_tensor("one16", [1, LI], BF16, kind="ExternalInput").ap()
    q0_d = bo_d = None
    if has_q0:
        q0_d = nc.dram_tensor("q0", [128, 8], F32, kind="ExternalInput").ap()
    if has_bo:
        bo_d = nc.dram_tensor("bo", [128, 4], F32, kind="ExternalInput").ap()
    y_d = nc.dram_tensor("y", [C, L], BF16, kind="ExternalOutput").ap()

    with tile.TileContext(nc) as tc:
        from contextlib import ExitStack

        with ExitStack() as ctx:
            cpool = ctx.enter_context(tc.tile_pool(name="consts", bufs=1))
            ps_s = ctx.enter_context(tc.tile_pool(name="ps_s", bufs=4, space="PSUM"))
            ps_z = ctx.enter_context(tc.tile_pool(name="ps_z", bufs=1, space="PSUM"))
            ps_u = ctx.enter_context(tc.tile_pool(name="ps_u", bufs=3, space="PSUM"))
            xpool = ctx.enter_context(tc.tile_pool(name="xp", bufs=2))
            ppool = ctx.enter_context(tc.tile_pool(name="pp", bufs=3))
            p8pool = ctx.enter_context(tc.tile_pool(name="p8p", bufs=2))
            opool = ctx.enter_context(tc.tile_pool(name="op", bufs=2))
            zpool = ctx.enter_context(tc.tile_pool(name="zp", bufs=2))

            # ---- loads in latency-priority order --------------------------
            one8_s = cpool.tile([128, 32], F8, name="one8s")
            nc.sync.dma_start(one8_s[:], one8_d[:, :])
            one16_s = cpool.tile([1, LI], BF16, name="one16s")
            nc.sync.dma_start(one16_s[:], one16_d[:, :])
            scale_imm = float(2.0 ** -kg)
            # gt ct=0 + ctx first halves unblock the jh=0 GC groups early.
            GTS = cpool.tile([128, 4 * C], BF16, name="gstk", tag="gstk")
            CTXT = cpool.tile([128, 4 * S], BF16, name="cstk", tag="cstk")
            nc.sync.dma_start(GTS[:, 0:512], gt_d[:, 0:512])
            for tt in range(4):
                nc.sync.dma_start(
                    CTXT[:, tt * S:tt * S + LI],
                    ctx_d[tt * 128:(tt + 1) * 128, 0:LI],
                )
            for ctb in range(1, 4):
                nc.sync.dma_start(
                    GTS[:, ctb * 512:(ctb + 1) * 512],
                    gt_d[:, ctb * 512:(ctb + 1) * 512],
                )

            def load_x(ic):
                xt = xpool.tile([128, 4 * LI], F8, name="xc", tag="x")
                nc.sync.dma_start(
                    xt[:].rearrange("p (t c) -> p t c", c=LI),
                    x_d[:, bass.ts(ic, LI)].rearrange("(t p) c -> p t c", p=128),
                )
                return xt

            X0 = load_x(0)                                        # prefetch chunk 0
            for tt in range(4):
                nc.sync.dma_start(
                    CTXT[:, tt * S + LI:(tt + 1) * S],
                    ctx_d[tt * 128:(tt + 1) * 128, LI:S],
                )
            WVT = cpool.tile([128, 4 * E], BF16, name="wstk", tag="wstk")
            for tt in range(4):
                nc.sync.dma_start(
                    WVT[:, tt * E:(tt + 1) * E],
                    wv_d[tt * 128:(tt + 1) * 128, :],
                )
            q0_s = bo_s = None
            if has_q0:
                q0_s = cpool.tile([128, 8], F32, name="q0s")
                nc.sync.dma_start(q0_s[:], q0_d[:, :])
            if has_bo:
                bo_s = cpool.tile([128, 4], F32, name="bos")
                nc.sync.dma_start(bo_s[:], bo_d[:, :])

            # ---- GC[c, j] = sum_e G[c, e] ctx[e, j]  (bf16, once) ---------
            # Output goes straight to the DoubleRow-interleaved fp8 layout:
            # GCD[cp][p, jt*256 + t*128 + m] = GC[(2cp+t)*128+p, jt*128+m].
            GCD = [
                cpool.tile([128, 2048], F8, name=f"gcd{cp}", tag=f"gcd{cp}")
                for cp in range(2)
            ]
            for jh in range(2):
                for ct in range(4):
                    gps = ps_s.tile([128, LI], F32, name="gps", tag="s")
                    for et in range(4):
                        nc.tensor.matmul(
                            gps[:],
                            GTS[:, ct * 512 + et * 128: ct * 512 + (et + 1) * 128],
                            CTXT[:, et * S + jh * LI: et * S + (jh + 1) * LI],
                            start=(et == 0),
                            stop=(et == 3),
                        )
                    dst = GCD[ct // 2][:, jh * 1024:(jh + 1) * 1024].rearrange(
                        "p (j two m) -> p j two m", two=2, m=128
                    )[:, :, ct % 2, :]
                    nc.vector.tensor_copy(
                        dst, gps[:].rearrange("p (j m) -> p j m", m=128)
                    )

            # ---- cbar[e] = sum_j ctx[e, j] ; Vbar = cbar @ WV (bf16) ------
            cbar = cpool.tile([128, 4], F32, name="cbar")
            nc.vector.tensor_reduce(
                cbar[:], CTXT[:].rearrange("p (t j) -> p t j", t=4), AXX, ADD
            )
            cbar16 = cpool.tile([128, 4], BF16, name="cbar16")
            nc.vector.tensor_copy(cbar16[:], cbar[:])

            def st_group(ic, jt, X, p8cur):
                """ST[j,i] for one j-tile: 2 DoubleRow fp8 matmuls, exp on
                scalar (with the 2^-kg descale), p8 = P - 1 cast on DVE."""
                sps = ps_s.tile([128, LI], F32, name="sps", tag="s")
                nc.tensor.matmul(
                    sps[:],
                    _two(GCD[0][:, jt * 256:(jt + 1) * 256], 128),
                    _two(X[:, 0:2 * LI], LI),
                    start=True,
                    stop=False,
                    perf_mode=DR,
                )
                nc.tensor.matmul(
                    sps[:],
                    _two(GCD[1][:, jt * 256:(jt + 1) * 256], 128),
                    _two(X[:, 2 * LI:4 * LI], LI),
                    start=False,
                    stop=True,
                    perf_mode=DR,
                )
                p = ppool.tile([128, LI], BF16, name="pt", tag=f"p{jt}")
                if has_q0:
                    nc.scalar.activation(
                        p[:], sps[:], EXP,
                        bias=q0_s[:, jt:jt + 1], scale=scale_imm,
                    )
                else:
                    nc.scalar.activation(p[:], sps[:], EXP, scale=scale_imm)
                jp, t = jt // 2, jt % 2
                if t == 0:
                    p8cur[jp] = p8pool.tile(
                        [128, 2 * LI], F8, name=f"p8_{jp}", tag=f"p8_{jp}"
                    )
                nc.vector.tensor_scalar_add(
                    p8cur[jp][:, t * LI:(t + 1) * LI], p[:], -1.0
                )

            def zsum_emit(p8prev):
                """Z - 1024 = sum_j p_j via DoubleRow ones-matmul, then the
                invz chain (DVE add/recip, gpsimd partition broadcast)."""
                zps = ps_z.tile([1, LI], F32, name="zps", tag="z")
                for jp in range(4):
                    nc.tensor.matmul(
                        zps[:],
                        _two(one8_s[:], 16)[:, :, 0:1],
                        _two(p8prev[jp][:], LI),
                        start=(jp == 0),
                        stop=(jp == 3),
                        perf_mode=DR,
                    )
                zr = zpool.tile([1, LI], F32, name="zr", tag="zr")
                nc.vector.tensor_scalar_add(zr[:], zps[:], 1024.0)
                invz = zpool.tile([1, LI], F32, name="invz", tag="invz")
                nc.vector.reciprocal_approx_fast(out=invz[:], in_=zr[:])
                invz_rep = zpool.tile([128, LI], F32, name="invzr", tag="invzr")
                nc.gpsimd.partition_broadcast(invz_rep[:], invz[:])
                return invz_rep

            def u_group(ic, ot, p8prev, VW8D, v16, invz_rep):
                """U[o,i] = Vbar[o] + sum_j p_j VW[j,o] (psum), then
                y = U * invz (DVE, bf16 out) and DMA out."""
                ups = ps_u.tile([128, LI], F32, name="ups", tag="u")
                nc.tensor.matmul(
                    ups[:],
                    v16[:, ot * 128:(ot + 1) * 128],
                    one16_s[:],
                    start=True,
                    stop=False,
                )
                for jp in range(4):
                    nc.tensor.matmul(
                        ups[:],
                        _two(VW8D[jp][:], E)[:, :, ot * 128:(ot + 1) * 128],
                        _two(p8prev[jp][:], LI),
                        start=False,
                        stop=(jp == 3),
                        perf_mode=DR,
                    )
                o = opool.tile([128, LI], BF16, name="ot", tag=f"o{ot}")
                nc.vector.tensor_mul(o[:], ups[:], invz_rep[:])
                if has_bo:
                    nc.vector.tensor_scalar_add(o[:], o[:], bo_s[:, ot:ot + 1])
                nc.sync.dma_start(y_d[ot * 128:(ot + 1) * 128, bass.ts(ic, LI)], o[:])

            # ---- window 0: ST(0), then VW + Vbar precompute ---------------
            X = X0
            Xnext = load_x(1)
            p8cur = {}
            st_group(0, 0, X, p8cur)
            st_group(0, 1, X, p8cur)
            # VW[j, o] = sum_e ctx[e, j] WV[e, o] (bf16, once), emitted after
            # the first ST groups so chunk 0's exp pipeline starts early; the
            # remaining ST groups interleave so sps psum slots recycle.
            VW8D = [None] * 4
            for jt in range(8):
                vps = ps_s.tile([128, E], F32, name="vps", tag="s")
                for et in range(4):
                    nc.tensor.matmul(
                        vps[:],
                        CTXT[:, et * S + jt * 128: et * S + (jt + 1) * 128],
                        WVT[:, et * E:(et + 1) * E],
                        start=(et == 0),
                        stop=(et == 3),
                    )
                jp, t = jt // 2, jt % 2
                if t == 0:
                    VW8D[jp] = cpool.tile(
                        [128, 2 * E], F8, name=f"vw8_{jp}", tag=f"vw8_{jp}"
                    )
                nc.vector.tensor_copy(VW8D[jp][:, t * E:(t + 1) * E], vps[:])
                if jt < 6:
                    st_group(0, jt + 2, X, p8cur)
            vb = ps_s.tile([1, E], F32, name="vb", tag="s")
            for et in range(4):
                nc.tensor.matmul(
                    vb[:],
                    cbar16[:, et:et + 1],
                    WVT[:, et * E:(et + 1) * E],
                    start=(et == 0),
                    stop=(et == 3),
                )
            v16 = cpool.tile([1, E], BF16, name="v16")
            nc.vector.tensor_copy(v16[:], vb[:])

            # ---- windows 1..8: ST(w) interleaved with U(w-1) --------------
            for w in range(1, NCHUNK + 1):
                p8prev, p8cur = p8cur, {}
                X, Xnext = Xnext, (load_x(w + 1) if w + 1 < NCHUNK else None)
                invz_rep = zsum_emit(p8prev)
                for k in range(4):
                    if w < NCHUNK:
                        st_group(w, 2 * k, X, p8cur)
                        st_group(w, 2 * k + 1, X, p8cur)
                    u_group(w - 1, k, p8prev, VW8D, v16, invz_rep)

    nc.compile()
    return nc


def kernel(**inputs) -> np.ndarray:
    global LAST_RESULTS
    x = np.asarray(inputs["x"], dtype=np.float32)
    context = np.asarray(inputs["context"], dtype=np.float32)
    W_pi = np.asarray(inputs["W_pi"], dtype=np.float64)
    b_pi = np.asarray(inputs["b_pi"], dtype=np.float64)
    W_q = np.asarray(inputs["W_q"], dtype=np.float64)
    b_q = np.asarray(inputs["b_q"], dtype=np.float64)
    W_k = np.asarray(inputs["W_k"], dtype=np.float64)
    W_v = np.asarray(inputs["W_v"], dtype=np.float64)
    b_v = np.asarray(inputs["b_v"], dtype=np.float64)
    W_po = np.asarray(inputs["W_po"], dtype=np.float64)
    b_po = np.asarray(inputs["b_po"], dtype=np.float64)

    scale = float(E) ** -0.5
    Wqpi = scale * (W_q @ W_pi)                            # [dq, c]
    G = (Wqpi.T @ W_k)                                     # [c, e]
    # fp8 pre-scale: |GC[c,j]| <= ||G[c,:]|| * ||ctx[:,j]|| and gaussian ctx
    # columns concentrate at sqrt(512)~22.6; 32 is a ~1.4x-margin bound.
    # Target max ~200 (e4m3 max normal is 240).
    rowg = float(np.linalg.norm(G, axis=1).max())
    kg = int(np.floor(np.log2(200.0 / (rowg * 32.0))))
    GT = np.ascontiguousarray(G.T * (2.0 ** kg)).astype(np.float32)   # [e, c]
    # ct-major block permutation: A[p, ct*512+et*128+c'] = GT[et*128+p, ct*128+c']
    GT = np.ascontiguousarray(
        GT.reshape(4, 128, 4, 128).transpose(1, 2, 0, 3).reshape(128, 4 * C)
    ).astype(NP_BF)
    b_row = scale * (W_q @ b_pi + b_q)
    q0_e = (W_k.T @ b_row).astype(np.float64)              # [e]
    WV = ((W_po @ W_v).T).astype(np.float32).astype(NP_BF)  # [e, o]
    b_o = (b_po + W_po @ b_v).astype(np.float32)           # [o]

    has_q0 = bool(np.any(q0_e))
    has_bo = bool(np.any(b_o))
    key = (has_q0, has_bo, kg)
    if key not in _PROGRAM_CACHE:
        _PROGRAM_CACHE[key] = _build_program(has_q0, has_bo, kg)
    nc = _PROGRAM_CACHE[key]

    one8 = np.ones((128, 32), dtype=NP_F8)
    one16 = np.ones((1, LI), dtype=NP_BF)
    in_maps = []
    for c in range(NCORES):
        ctx_mat = context[c].reshape(E, S)
        m = {
            "x": x[c].reshape(C, L).astype(NP_F8),
            "ctx": ctx_mat.astype(NP_BF),
            "gt": GT,
            "wv": WV,
            "one8": one8,
            "one16": one16,
        }
        if has_q0:
            # logits bias per key j: q0_e . ctx[:, j]  -> [S] -> [128, 8]
            q0j = (q0_e @ ctx_mat.astype(np.float64)).astype(np.float32)
            m["q0"] = np.ascontiguousarray(q0j.reshape(8, 128).T)
        if has_bo:
            m["bo"] = np.ascontiguousarray(b_o.reshape(4, 128).T)
        in_maps.append(m)

    res = run_bass_kernel_spmd(nc, in_maps, core_ids=list(range(NCORES)), trace=TRACE)
    LAST_RESULTS = res
    y = np.stack(
        [np.asarray(res.results[c]["y"]).astype(np.float32) for c in range(NCORES)],
        axis=0,
    )
    return np.ascontiguousarray(y.reshape(NCORES, C, 64, 64))


# revision 15
# speedup vs baseline: 1.3255x; 1.0608x over previous
"""Cross-attention kernel for TRN2, data-parallel over batch (B=8) on 8 cores.

Reference computation per batch element:
    xt  = proj_in(x)              # [L=4096, E=512], 1x1 conv == matmul
    Q   = xt @ W_q.T + b_q
    K   = ctx @ W_k.T + b_k       # ctx: [S=1024, E]
    V   = ctx @ W_v.T + b_v
    att = softmax(Q @ K.T * scale)
    out = proj_out((att @ V).T)   # [C=512, 64, 64]

Host-side algebraic folds (weights only, exact up to fp rounding):
  * scale, W_pi, W_q, W_k fold into G = (scale * W_q @ W_pi).T @ W_k, so
    logits.T = (G.T-contract ctx).T-contract X.  G is pre-scaled by 2^kg
    (weights-only bound) so its fp8 products stay in e4m3 normal range;
    the inverse scale rides the exp() activation's scale operand.
  * W_v and W_po fold:  WV = (W_po @ W_v).T ; b_o = b_po + W_po @ b_v

The two big attention GEMMs (logits ST = GC.T-c X and output U = VW.T-c PT)
run as fp8e4m3 DoubleRow matmuls: 256 contraction rows per instruction at
0.5 cycles/row -- 2x the bf16/fp32r MAC rate.  fp8's 3-bit mantissa cannot
represent softmax weights P ~= 1 +- 0.1 (quantization step 0.125 at 1.0),
so the softmax is mean-split:  P = 1 + p,  U = Vbar + sum_j p_j VW_j with
p = exp(s) - 1 cast to fp8 (full relative resolution on the deviation) and
Vbar = sum_j VW_j computed via a bf16 path (cbar = rowsum(ctx);
Vbar = cbar @ WV) so the dominant mean term carries no fp8 noise.
Z = 1024 + sum_j p_j from a DoubleRow ones-matmul over the same p8 tiles
(consistent with the numerator).

Per-core precomputes GC = G.T-c ctx and VW = ctx.T-c WV stay bf16 (fp8
operands there cost ~0.5e-2 extra max-norm error for only ~7us).

Measured end-to-end max-norm rel err of this quantization pipeline vs the
fp64 reference: ~1.1e-2 (budget 2e-2).
"""

import numpy as np
import ml_dtypes

import concourse.bass as bass
import concourse.mybir as mybir
import concourse.tile as tile
from concourse import bacc
from concourse.bass_utils import run_bass_kernel_spmd

F32 = mybir.dt.float32
BF16 = mybir.dt.bfloat16
F8 = mybir.dt.float8e4
EXP = mybir.ActivationFunctionType.Exp
DR = mybir.MatmulPerfMode.DoubleRow
AXX = mybir.AxisListType.X
ADD = mybir.AluOpType.add

NP_F8 = ml_dtypes.float8_e4m3
NP_BF = ml_dtypes.bfloat16

C = 512       # in channels
E = 512       # emb dim
L = 4096      # query length (64*64)
S = 1024      # key length (32*32)
LI = 512      # i-chunk (query) tile size
NCHUNK = L // LI
NCORES = 8

TRACE = False           # test harness can flip this before calling kernel()
LAST_RESULTS = None     # stashed BassKernelResults for the test harness

_PROGRAM_CACHE = {}


def _two(ap, inner):
    """[128, 2*inner] AP -> [128, 2, inner] for DoubleRow operands."""
    return ap.rearrange("p (two n) -> p two n", two=2, n=inner)


def _build_program(has_q0: bool, has_bo: bool, kg: int):
    nc = bacc.Bacc(
        "TRN2",
        target_bir_lowering=False,
        debug=False,
        enable_asserts=False,
        num_devices=NCORES,
    )
    x_d = nc.dram_tensor("x", [C, L], F8, kind="ExternalInput").ap()
    ctx_d = nc.dram_tensor("ctx", [E, S], BF16, kind="ExternalInput").ap()
    # gt arrives host-permuted into ct-major blocks: gt_d[p, ct*512+et*128+c']
    # = (2^kg * G.T)[et*128+p, ct*128+c'].
    gt_d = nc.dram_tensor("gt", [128, 4 * C], BF16, kind="ExternalInput").ap()
    wv_d = nc.dram_tensor("wv", [E, E], BF16, kind="ExternalInput").ap()
    # [128, 32] of ones; the Sigma-p DoubleRow lhsT reads cols {0, 16} so the
    # pair stride is 16B (dual-fp8 ldweights requires 16B-aligned even steps).
    one8_d = nc.dram_tensor("one8", [128, 32], F8, kind="ExternalInput").ap()
    one16_d = nc.dram_tensor("one16", [1, LI], BF16, kind="ExternalInput").ap()
    q0_d = bo_d = None
    if has_q0:
        q0_d = nc.dram_tensor("q0", [128, 8], F32, kind="ExternalInput").ap()
    if has_bo:
        bo_d = nc.dram_tensor("bo", [128, 4], F32, kind="ExternalInput").ap()
    y_d = nc.dram_tensor("y", [C, L], BF16, kind="ExternalOutput").ap()

    with tile.TileContext(nc) as tc:
        from contextlib import ExitStack

        with ExitStack() as ctx:
            cpool = ctx.enter_context(tc.tile_pool(name="consts", bufs=1))
            ps_s = ctx.enter_context(tc.tile_pool(name="ps_s", bufs=4, space="PSUM"))
            ps_z = ctx.enter_context(tc.tile_pool(name="ps_z", bufs=1, space="PSUM"))
            ps_u = ctx.enter_context(tc.tile_pool(name="ps_u", bufs=3, space="PSUM"))
            xpool = ctx.enter_context(tc.tile_pool(name="xp", bufs=2))
            ppool = ctx.enter_context(tc.tile_pool(name="pp", bufs=3))
            p8pool = ctx.enter_context(tc.tile_pool(name="p8p", bufs=2))
            opool = ctx.enter_context(tc.tile_pool(name="op", bufs=2))
            zpool = ctx.enter_context(tc.tile_pool(name="zp", bufs=2))

            # ---- loads in latency-priority order --------------------------
            one8_s = cpool.tile([128, 32], F8, name="one8s")
            nc.sync.dma_start(one8_s[:], one8_d[:, :])
            one16_s = cpool.tile([1, LI], BF16, name="one16s")
            nc.sync.dma_start(one16_s[:], one16_d[:, :])
            scale_imm = float(2.0 ** -kg)
            # gt ct=0 + ctx first halves unblock the jh=0 GC groups early.
            GTS = cpool.tile([128, 4 * C], BF16, name="gstk", tag="gstk")
            CTXT = cpool.tile([128, 4 * S], BF16, name="cstk", tag="cstk")
            nc.sync.dma_start(GTS[:, 0:512], gt_d[:, 0:512])
            for tt in range(4):
                nc.sync.dma_start(
                    CTXT[:, tt * S:tt * S + LI],
                    ctx_d[tt * 128:(tt + 1) * 128, 0:LI],
                )
            for ctb in range(1, 4):
                nc.sync.dma_start(
                    GTS[:, ctb * 512:(ctb + 1) * 512],
                    gt_d[:, ctb * 512:(ctb + 1) * 512],
                )

            def load_x(ic):
                xt = xpool.tile([128, 4 * LI], F8, name="xc", tag="x")
                nc.sync.dma_start(
                    xt[:].rearrange("p (t c) -> p t c", c=LI),
                    x_d[:, bass.ts(ic, LI)].rearrange("(t p) c -> p t c", p=128),
                )
                return xt

            X0 = load_x(0)                                        # prefetch chunk 0
            for tt in range(4):
                nc.sync.dma_start(
                    CTXT[:, tt * S + LI:(tt + 1) * S],
                    ctx_d[tt * 128:(tt + 1) * 128, LI:S],
                )
            WVT = cpool.tile([128, 4 * E], BF16, name="wstk", tag="wstk")
            for tt in range(4):
                nc.sync.dma_start(
                    WVT[:, tt * E:(tt + 1) * E],
                    wv_d[tt * 128:(tt + 1) * 128, :],
                )
            q0_s = bo_s = None
            if has_q0:
                q0_s = cpool.tile([128, 8], F32, name="q0s")
                nc.sync.dma_start(q0_s[:], q0_d[:, :])
            if has_bo:
                bo_s = cpool.tile([128, 4], F32, name="bos")
                nc.sync.dma_start(bo_s[:], bo_d[:, :])

            # ---- GC[c, j] = sum_e G[c, e] ctx[e, j]  (bf16, once) ---------
            # Output goes straight to the DoubleRow-interleaved fp8 layout:
            # GCD[cp][p, jt*256 + t*128 + m] = GC[(2cp+t)*128+p, jt*128+m].
            GCD = [
                cpool.tile([128, 2048], F8, name=f"gcd{cp}", tag=f"gcd{cp}")
                for cp in range(2)
            ]
            for jh in range(2):
                for ct in range(4):
                    gps = ps_s.tile([128, LI], F32, name="gps", tag="s")
                    for et in range(4):
                        nc.tensor.matmul(
                            gps[:],
                            GTS[:, ct * 512 + et * 128: ct * 512 + (et + 1) * 128],
                            CTXT[:, et * S + jh * LI: et * S + (jh + 1) * LI],
                            start=(et == 0),
                            stop=(et == 3),
                        )
                    dst = GCD[ct // 2][:, jh * 1024:(jh + 1) * 1024].rearrange(
                        "p (j two m) -> p j two m", two=2, m=128
                    )[:, :, ct % 2, :]
                    nc.vector.tensor_copy(
                        dst, gps[:].rearrange("p (j m) -> p j m", m=128)
                    )

            # ---- cbar[e] = sum_j ctx[e, j] ; Vbar = cbar @ WV (bf16) ------
            cbar = cpool.tile([128, 4], F32, name="cbar")
            nc.vector.tensor_reduce(
                cbar[:], CTXT[:].rearrange("p (t j) -> p t j", t=4), AXX, ADD
            )
            cbar16 = cpool.tile([128, 4], BF16, name="cbar16")
            nc.vector.tensor_copy(cbar16[:], cbar[:])

            def st_group(ic, jt, X, p8cur):
                """ST[j,i] for one j-tile: 2 DoubleRow fp8 matmuls, exp on
                scalar (with the 2^-kg descale), p8 = P - 1 cast on DVE."""
                sps = ps_s.tile([128, LI], F32, name="sps", tag="s")
                nc.tensor.matmul(
                    sps[:],
                    _two(GCD[0][:, jt * 256:(jt + 1) * 256], 128),
                    _two(X[:, 0:2 * LI], LI),
                    start=True,
                    stop=False,
                    perf_mode=DR,
                )
                nc.tensor.matmul(
                    sps[:],
                    _two(GCD[1][:, jt * 256:(jt + 1) * 256], 128),
                    _two(X[:, 2 * LI:4 * LI], LI),
                    start=False,
                    stop=True,
                    perf_mode=DR,
                )
                p = ppool.tile([128, LI], BF16, name="pt", tag=f"p{jt}")
                if has_q0:
                    nc.scalar.activation(
                        p[:], sps[:], EXP,
                        bias=q0_s[:, jt:jt + 1], scale=scale_imm,
                    )
                else:
                    nc.scalar.activation(p[:], sps[:], EXP, scale=scale_imm)
                jp, t = jt // 2, jt % 2
                if t == 0:
                    p8cur[jp] = p8pool.tile(
                        [128, 2 * LI], F8, name=f"p8_{jp}", tag=f"p8_{jp}"
                    )
                nc.vector.tensor_scalar_add(
                    p8cur[jp][:, t * LI:(t + 1) * LI], p[:], -1.0
                )

            def zsum_emit(p8prev):
                """Z - 1024 = sum_j p_j via DoubleRow ones-matmul, then the
                invz chain (DVE add/recip, gpsimd partition broadcast)."""
                zps = ps_z.tile([1, LI], F32, name="zps", tag="z")
                for jp in range(4):
                    nc.tensor.matmul(
                        zps[:],
                        _two(one8_s[:], 16)[:, :, 0:1],
                        _two(p8prev[jp][:], LI),
                        start=(jp == 0),
                        stop=(jp == 3),
                        perf_mode=DR,
                    )
                zr = zpool.tile([1, LI], F32, name="zr", tag="zr")
                nc.vector.tensor_scalar_add(zr[:], zps[:], 1024.0)
                invz = zpool.tile([1, LI], F32, name="invz", tag="invz")
                nc.vector.reciprocal_approx_fast(out=invz[:], in_=zr[:])
                invz_rep = zpool.tile([128, LI], F32, name="invzr", tag="invzr")
                nc.gpsimd.partition_broadcast(invz_rep[:], invz[:])
                return invz_rep

            def u_group(ic, ot, p8prev, VW8D, v16, invz_rep):
                """U[o,i] = Vbar[o] + sum_j p_j VW[j,o] (psum), then
                y = U * invz (DVE, bf16 out) and DMA out."""
                ups = ps_u.tile([128, LI], F32, name="ups", tag="u")
                nc.tensor.matmul(
                    ups[:],
                    v16[:, ot * 128:(ot + 1) * 128],
                    one16_s[:],
                    start=True,
                    stop=False,
                )
                for jp in range(4):
                    nc.tensor.matmul(
                        ups[:],
                        _two(VW8D[jp][:], E)[:, :, ot * 128:(ot + 1) * 128],
                        _two(p8prev[jp][:], LI),
                        start=False,
                        stop=(jp == 3),
                        perf_mode=DR,
                    )
                o = opool.tile([128, LI], BF16, name="ot", tag=f"o{ot}")
                nc.vector.tensor_mul(o[:], ups[:], invz_rep[:])
                if has_bo:
                    nc.vector.tensor_scalar_add(o[:], o[:], bo_s[:, ot:ot + 1])
                nc.sync.dma_start(y_d[ot * 128:(ot + 1) * 128, bass.ts(ic, LI)], o[:])

            # ---- window 0: ST(0), then VW + Vbar precompute ---------------
            X = X0
            Xnext = load_x(1)
            p8cur = {}
            st_group(0, 0, X, p8cur)
            st_group(0, 1, X, p8cur)
            # VW[j, o] = sum_e ctx[e, j] WV[e, o] (bf16, once), emitted after
            # the first ST groups so chunk 0's exp pipeline starts early; the
            # remaining ST groups interleave so sps psum slots recycle.
            VW8D = [None] * 4
            for jt in range(8):
                vps = ps_s.tile([128, E], F32, name="vps", tag="s")
                for et in range(4):
                    nc.tensor.matmul(
                        vps[:],
                        CTXT[:, et * S + jt * 128: et * S + (jt + 1) * 128],
                        WVT[:, et * E:(et + 1) * E],
                        start=(et == 0),
                        stop=(et == 3),
                    )
                jp, t = jt // 2, jt % 2
                if t == 0:
                    VW8D[jp] = cpool.tile(
                        [128, 2 * E], F8, name=f"vw8_{jp}", tag=f"vw8_{jp}"
                    )
                nc.vector.tensor_copy(VW8D[jp][:, t * E:(t + 1) * E], vps[:])
                if jt < 6:
                    st_group(0, jt + 2, X, p8cur)
            vb = ps_s.tile([1, E], F32, name="vb", tag="s")
            for et in range(4):
                nc.tensor.matmul(
                    vb[:],
                    cbar16[:, et:et + 1],
                    WVT[:, et * E:(et + 1) * E],
                    start=(et == 0),
                    stop=(et == 3),
                )
            v16 = cpool.tile([1, E], BF16, name="v16")
            nc.vector.tensor_copy(v16[:], vb[:])

            # ---- windows 1..8: ST(w) interleaved with U(w-1) --------------
            for w in range(1, NCHUNK + 1):
                p8prev, p8cur = p8cur, {}
                X, Xnext = Xnext, (load_x(w + 1) if w + 1 < NCHUNK else None)
                # Sigma-p(w-1) depends on the LAST cast of the previous chunk;
                # emitting the first ST pair ahead of it keeps the PE queue
                # fed with ready work at the window boundary (p-state ramp).
                invz_rep = None
                for k in range(4):
                    if w < NCHUNK:
                        st_group(w, 2 * k, X, p8cur)
                        st_group(w, 2 * k + 1, X, p8cur)
                    if k == 0:
                        invz_rep = zsum_emit(p8prev)
                    u_group(w - 1, k, p8prev, VW8D, v16, invz_rep)

    nc.compile()
    return nc


def kernel(**inputs) -> np.ndarray:
    global LAST_RESULTS
    x = np.asarray(inputs["x"], dtype=np.float32)
    context = np.asarray(inputs["context"], dtype=np.float32)
    W_pi = np.asarray(inputs["W_pi"], dtype=np.float64)
    b_pi = np.asarray(inputs["b_pi"], dtype=np.float64)
    W_q = np.asarray(inputs["W_q"], dtype=np.float64)
    b_q = np.asarray(inputs["b_q"], dtype=np.float64)
    W_k = np.asarray(inputs["W_k"], dtype=np.float64)
    W_v = np.asarray(inputs["W_v"], dtype=np.float64)
    b_v = np.asarray(inputs["b_v"], dtype=np.float64)
    W_po = np.asarray(inputs["W_po"], dtype=np.float64)
    b_po = np.asarray(inputs["b_po"], dtype=np.float64)

    scale = float(E) ** -0.5
    Wqpi = scale * (W_q @ W_pi)                            # [dq, c]
    G = (Wqpi.T @ W_k)                                     # [c, e]
    # fp8 pre-scale: |GC[c,j]| <= ||G[c,:]|| * ||ctx[:,j]|| and gaussian ctx
    # columns concentrate at sqrt(512)~22.6; 32 is a ~1.4x-margin bound.
    # Target max ~200 (e4m3 max normal is 240).
    rowg = float(np.linalg.norm(G, axis=1).max())
    kg = int(np.floor(np.log2(200.0 / (rowg * 32.0))))
    GT = np.ascontiguousarray(G.T * (2.0 ** kg)).astype(np.float32)   # [e, c]
    # ct-major block permutation: A[p, ct*512+et*128+c'] = GT[et*128+p, ct*128+c']
    GT = np.ascontiguousarray(
        GT.reshape(4, 128, 4, 128).transpose(1, 2, 0, 3).reshape(128, 4 * C)
    ).astype(NP_BF)
    b_row = scale * (W_q @ b_pi + b_q)
    q0_e = (W_k.T @ b_row).astype(np.float64)              # [e]
    WV = ((W_po @ W_v).T).astype(np.float32).astype(NP_BF)  # [e, o]
    b_o = (b_po + W_po @ b_v).astype(np.float32)           # [o]

    has_q0 = bool(np.any(q0_e))
    has_bo = bool(np.any(b_o))
    key = (has_q0, has_bo, kg)
    if key not in _PROGRAM_CACHE:
        _PROGRAM_CACHE[key] = _build_program(has_q0, has_bo, kg)
    nc = _PROGRAM_CACHE[key]

    one8 = np.ones((128, 32), dtype=NP_F8)
    one16 = np.ones((1, LI), dtype=NP_BF)
    in_maps = []
    for c in range(NCORES):
        ctx_mat = context[c].reshape(E, S)
        m = {
            "x": x[c].reshape(C, L).astype(NP_F8),
            "ctx": ctx_mat.astype(NP_BF),
            "gt": GT,
            "wv": WV,
            "one8": one8,
            "one16": one16,
        }
        if has_q0:
            # logits bias per key j: q0_e . ctx[:, j]  -> [S] -> [128, 8]
            q0j = (q0_e @ ctx_mat.astype(np.float64)).astype(np.float32)
            m["q0"] = np.ascontiguousarray(q0j.reshape(8, 128).T)
        if has_bo:
            m["bo"] = np.ascontiguousarray(b_o.reshape(4, 128).T)
        in_maps.append(m)

    res = run_bass_kernel_spmd(nc, in_maps, core_ids=list(range(NCORES)), trace=TRACE)
    LAST_RESULTS = res
    y = np.stack(
        [np.asarray(res.results[c]["y"]).astype(np.float32) for c in range(NCORES)],
        axis=0,
    )
    return np.ascontiguousarray(y.reshape(NCORES, C, 64, 64))
